# revision 1
# baseline (speedup 1.0000x reference)
"""DRC-GCN (8-layer GNN message passing) on 8 trn2 NeuronCores.

Strategy (node/data parallel):
  - Destination nodes sharded across 8 cores (12500 -> padded 12544 rows each).
  - Per layer: every core gathers source rows from a replicated bf16 copy of
    the full hidden matrix M (AllGather'd each layer), computes its shard of
    AX = spmm(M) via dma_gather + "staircase" segment-sum matmuls
    (lhsT = per-chunk selection matrix SegW[slot, dest] = w_e * (dest_e == dest)),
    then the dense per-layer update in fp32 on a transposed accumulator.
  - Graph structure is preprocessed on host: edges sorted per (core, dest
    block of 128, source bucket of 16768 rows), padded to static capacities,
    gathered with int16 indices (dma_gather), slot metadata (local dest +
    edge weight) drives the SegW build on DVE.
"""

import math

import numpy as np
import ml_dtypes

import concourse.bass as bass
import concourse.mybir as mybir
import concourse.tile as tile
from concourse import bacc
from concourse import bass_utils

FP32 = mybir.dt.float32
BF16 = mybir.dt.bfloat16
I16 = mybir.dt.int16
I32 = mybir.dt.int32
AF = mybir.ActivationFunctionType
ALU = mybir.AluOpType

P = 128
CORES = 8
TAU = 0.5
BUCKET = 16768          # gather bucket rows (int16-addressable, mult of 128)
NI_MAX = 1024           # dma_gather num_idxs hard limit on trn2
NQ = 4                  # SWDGE queues


def _round_up(x, m):
    return (x + m - 1) // m * m


class Plan:
    """Static (core-invariant) kernel structure + per-core data arrays."""

    def __init__(self, n_nodes, nfeat, nhid, ncls, nlayers,
                 edge_row, edge_col, edge_w):
        self.n = n_nodes
        self.nfeat = nfeat
        self.nhid = nhid
        self.ncls = ncls
        self.nl = nlayers
        assert n_nodes % CORES == 0
        self.nsh_raw = n_nodes // CORES
        self.nsh = _round_up(self.nsh_raw, P)
        self.nb = self.nsh // P
        self.ntot = self.nsh * CORES
        self.nbuck = (self.ntot + BUCKET - 1) // BUCKET

        # global padded source id
        own = edge_col // self.nsh_raw
        gsrc = own * self.nsh + (edge_col - own * self.nsh_raw)
        dst_core = edge_row // self.nsh_raw
        ldst = edge_row - dst_core * self.nsh_raw          # local dest
        blk = ldst // P
        ldst_in_blk = ldst % P
        bucket = gsrc // BUCKET
        lidx = gsrc - bucket * BUCKET                      # int16 index

        # per (core, block, bucket) segment sort
        counts = np.zeros((CORES, self.nb, self.nbuck), np.int64)
        np.add.at(counts, (dst_core, blk, bucket), 1)
        # static capacities: max over cores, rounded to 128
        capb = _round_up(counts.max(axis=0), P)            # [nb, nbuck]
        self.capb = capb
        self.ch = capb.sum(axis=1) // P                    # chunks per block
        self.chmax = int(self.ch.max())
        self.oc = np.concatenate([[0], np.cumsum(self.ch)]).astype(np.int64)
        self.totch = int(self.oc[-1])

        # gather call list per block: (bucket, chunk_off_in_block, n_idx, idx16_off)
        self.calls = []
        o16 = 0
        for b in range(self.nb):
            cblk = []
            co = 0
            for k in range(self.nbuck):
                cap = int(capb[b, k])
                left = cap
                while left > 0:
                    ni = min(left, NI_MAX)
                    # keep pieces balanced multiples of 128
                    if left > NI_MAX:
                        ni = _round_up(left // 2, P)
                        ni = min(ni, NI_MAX)
                    cblk.append((k, co, ni, o16))
                    co += ni // P
                    o16 += ni // 16
                    left -= ni
            self.calls.append(cblk)
        self.tot16 = o16

        # --------- per-core data arrays ---------
        order = np.lexsort((lidx, bucket, blk, dst_core))
        self.eidx = np.zeros((CORES, P, self.tot16), np.int16)
        self.eld = np.full((CORES, P, self.totch), -1.0, np.float32)
        self.ew = np.zeros((CORES, P, self.totch), np.float32)

        srt_core = dst_core[order]
        srt_blk = blk[order]
        srt_bucket = bucket[order]
        srt_lidx = lidx[order]
        srt_ld = ldst_in_blk[order]
        srt_w = edge_w[order]

        core_starts = np.searchsorted(srt_core, np.arange(CORES + 1))
        for c in range(CORES):
            s0, s1 = core_starts[c], core_starts[c + 1]
            cblk = srt_blk[s0:s1]
            cbuck = srt_bucket[s0:s1]
            clidx = srt_lidx[s0:s1]
            cld = srt_ld[s0:s1]
            cw = srt_w[s0:s1]
            # segment starts in the sorted stream
            seg_key = cblk * self.nbuck + cbuck
            seg_starts = np.searchsorted(seg_key, np.arange(self.nb * self.nbuck + 1))
            for b in range(self.nb):
                co_base = 0
                for k in range(self.nbuck):
                    a0 = seg_starts[b * self.nbuck + k]
                    a1 = seg_starts[b * self.nbuck + k + 1]
                    nreal = a1 - a0
                    cap = int(capb[b, k])
                    assert nreal <= cap
                    vals = np.zeros(cap, np.int16)
                    vals[:nreal] = clidx[a0:a1]
                    lds = np.full(cap, -1.0, np.float32)
                    lds[:nreal] = cld[a0:a1]
                    ws = np.zeros(cap, np.float32)
                    ws[:nreal] = cw[a0:a1]
                    # slots -> (partition i%128, chunk co_base + i//128)
                    jj = np.arange(cap)
                    self.eld[c, jj % P, self.oc[b] + co_base + jj // P] = lds
                    self.ew[c, jj % P, self.oc[b] + co_base + jj // P] = ws
                    co_base += cap // P
        # idx packing (per call, per core)
        for c in range(CORES):
            s0, s1 = core_starts[c], core_starts[c + 1]
            cblk = srt_blk[s0:s1]
            cbuck = srt_bucket[s0:s1]
            clidx = srt_lidx[s0:s1]
            seg_key = cblk * self.nbuck + cbuck
            seg_starts = np.searchsorted(seg_key, np.arange(self.nb * self.nbuck + 1))
            for b in range(self.nb):
                # per bucket padded index stream
                consumed = {k: 0 for k in range(self.nbuck)}
                for (k, co, ni, o16) in self.calls[b]:
                    a0 = seg_starts[b * self.nbuck + k]
                    a1 = seg_starts[b * self.nbuck + k + 1]
                    seg = np.zeros(int(self.capb[b, k]), np.int16)
                    seg[:a1 - a0] = clidx[a0:a1]
                    piece = seg[consumed[k]:consumed[k] + ni]
                    consumed[k] += ni
                    wr = piece.reshape(ni // 16, 16).T          # [16, ni/16]
                    self.eidx[c, :, o16:o16 + ni // 16] = np.tile(wr, (8, 1))


def build_nc(plan: Plan):
    nl, nb, nsh, ntot = plan.nl, plan.nb, plan.nsh, plan.ntot
    nhid, ncls, nfeat = plan.nhid, plan.ncls, plan.nfeat
    nh2 = nhid // P          # hidden partition-halves (2)
    nf2 = nfeat // P         # feat chunks (4)

    nc = bacc.Bacc("TRN2", target_bir_lowering=False, debug=False,
                   enable_asserts=True, num_devices=CORES, num_swdge_queues=NQ)

    xt = nc.dram_tensor("xt", [nfeat, nsh], FP32, kind="ExternalInput").ap()
    wi = nc.dram_tensor("wi", [nfeat, nhid], FP32, kind="ExternalInput").ap()
    bi = nc.dram_tensor("bi", [P, nh2], FP32, kind="ExternalInput").ap()
    wm = nc.dram_tensor("wm", [nl, nhid, nhid], FP32, kind="ExternalInput").ap()
    gm = nc.dram_tensor("gm", [P, nl], FP32, kind="ExternalInput").ap()
    ws = nc.dram_tensor("ws", [nhid, ncls], FP32, kind="ExternalInput").ap()
    bs = nc.dram_tensor("bs", [ncls, 1], FP32, kind="ExternalInput").ap()
    eidx = nc.dram_tensor("eidx", [P, plan.tot16], I16, kind="ExternalInput").ap()
    eld = nc.dram_tensor("eld", [P, plan.totch], FP32, kind="ExternalInput").ap()
    ew = nc.dram_tensor("ew", [P, plan.totch], FP32, kind="ExternalInput").ap()
    out = nc.dram_tensor("out", [nsh, ncls], FP32, kind="ExternalOutput").ap()

    with tile.TileContext(nc) as tc:
        with tc.tile_pool(name="c1", bufs=1) as c1, \
             tc.tile_pool(name="stream", bufs=3) as st, \
             tc.tile_pool(name="gpool", bufs=2) as gp, \
             tc.tile_pool(name="wpool", bufs=2) as wp, \
             tc.tile_pool(name="ps", bufs=2, space="PSUM") as ps, \
             tc.tile_pool(name="psax", bufs=2, space="PSUM") as psax, \
             tc.tile_pool(name="dram", bufs=1, space="DRAM") as dp:

            # ---------- constants ----------
            iota_i = c1.tile([P, P], I32)
            nc.gpsimd.iota(iota_i[:], pattern=[[1, P]], base=0, channel_multiplier=0)
            iotap_i = c1.tile([P, 1], I32)
            nc.gpsimd.iota(iotap_i[:], pattern=[[1, 1]], base=0, channel_multiplier=1)
            iota_f = c1.tile([P, P], FP32)
            nc.vector.tensor_copy(out=iota_f[:], in_=iota_i[:])
            iotap_f = c1.tile([P, 1], FP32)
            nc.vector.tensor_copy(out=iotap_f[:], in_=iotap_i[:])
            ident = c1.tile([P, P], FP32)
            nc.vector.tensor_tensor(out=ident[:], in0=iota_f[:],
                                    in1=iotap_f[:].to_broadcast([P, P]),
                                    op=ALU.is_equal)

            wi_sb = c1.tile([P, nf2, nhid], FP32)
            for kc in range(nf2):
                nc.sync.dma_start(out=wi_sb[:, kc, :], in_=wi[kc * P:(kc + 1) * P, :])
            bi_sb = c1.tile([P, nh2], FP32)
            nc.sync.dma_start(out=bi_sb[:], in_=bi[:])
            ws_sb = c1.tile([P, nh2, ncls], FP32)
            for h in range(nh2):
                nc.sync.dma_start(out=ws_sb[:, h, :], in_=ws[h * P:(h + 1) * P, :])
            bs_sb = c1.tile([ncls, 1], FP32)
            nc.sync.dma_start(out=bs_sb[:], in_=bs[:])
            gm_sb = c1.tile([P, nl], FP32)
            nc.sync.dma_start(out=gm_sb[:], in_=gm[:])

            accT = [c1.tile([P, nsh], FP32, name=f"accT{h}") for h in range(nh2)]

            # DRAM intermediates: one Shared AllGather output per layer
            # (Shared scratchpad tensors allow only a single writer inst)
            Ms = [dp.tile([ntot, nhid], BF16, addr_space="Shared", name=f"M{l}")
                  for l in range(nl)]
            xc0 = dp.tile([nsh, nhid], FP32, name="xc0")
            xc1 = dp.tile([nsh, nhid], FP32, name="xc1")
            xcs = [xc0, xc1]

            # ---------- init: H = X @ Wi + bi ----------
            agin = dp.tile([nsh, nhid], BF16, name="agin0", bufs=2, tag="agin")
            for b in range(nb):
                cols = slice(b * P, (b + 1) * P)
                xt_sb = st.tile([P, nf2, P], FP32, tag="xt")
                for kc in range(nf2):
                    nc.sync.dma_start(out=xt_sb[:, kc, :],
                                      in_=xt[kc * P:(kc + 1) * P, cols])
                hst = st.tile([P, nhid], FP32, tag="hst")
                for h in range(nh2):
                    hps = ps.tile([P, P], FP32, tag="mm")
                    for kc in range(nf2):
                        nc.tensor.matmul(hps[:], lhsT=wi_sb[:, kc, h * P:(h + 1) * P],
                                         rhs=xt_sb[:, kc, :],
                                         start=(kc == 0), stop=(kc == nf2 - 1))
                    # accT init with bias
                    nc.vector.tensor_scalar(out=accT[h][:, cols], in0=hps[:],
                                            scalar1=bi_sb[:, h:h + 1], scalar2=None,
                                            op0=ALU.add)
                    # transpose to straight layout
                    tps = ps.tile([P, P], FP32, tag="tr")
                    nc.tensor.transpose(out=tps[:], in_=accT[h][:, cols],
                                        identity=ident[:])
                    nc.scalar.activation(out=hst[:, h * P:(h + 1) * P], in_=tps[:],
                                         func=AF.Copy)
                nc.sync.dma_start(out=xcs[0][cols, :], in_=hst[:])
                hbf = st.tile([P, nhid], BF16, tag="hbf")
                nc.vector.tensor_copy(out=hbf[:], in_=hst[:])
                nc.scalar.dma_start(out=agin[cols, :], in_=hbf[:])
            nc.gpsimd.collective_compute(
                "AllGather", ALU.bypass,
                replica_groups=[list(range(CORES))],
                ins=[agin[:]], outs=[Ms[0][:]],
            )

            # ---------- layers ----------
            qrot = 0
            for l in range(nl):
                Mcur = Ms[l]
                Mnxt = Ms[l + 1] if l + 1 < nl else None
                xprev = xcs[l % 2]
                xnext = xcs[(l + 1) % 2]
                last = (l == nl - 1)

                wm_sb = wp.tile([P, nh2, nhid], FP32, tag="wm")
                for kc in range(nh2):
                    nc.sync.dma_start(out=wm_sb[:, kc, :],
                                      in_=wm[l, kc * P:(kc + 1) * P, :])
                if not last:
                    agin = dp.tile([nsh, nhid], BF16, name=f"agin{l+1}",
                                   bufs=2, tag="agin")

                for b in range(nb):
                    cols = slice(b * P, (b + 1) * P)
                    ch = int(plan.ch[b])
                    oc = int(plan.oc[b])
                    # metadata (ch*128/16 == ch*8 int16 columns for this block)
                    idx_t = st.tile([P, ch * 8], I16, tag="idx")
                    o16b = plan.calls[b][0][3] if plan.calls[b] else 0
                    nc.sync.dma_start(out=idx_t[:],
                                      in_=eidx[:, o16b:o16b + ch * 8])
                    ld_t = st.tile([P, ch], FP32, tag="ld")
                    nc.sync.dma_start(out=ld_t[:], in_=eld[:, oc:oc + ch])
                    w_t = st.tile([P, ch], FP32, tag="w")
                    nc.sync.dma_start(out=w_t[:], in_=ew[:, oc:oc + ch])

                    G = gp.tile([P, plan.chmax, nhid], BF16, tag="g")
                    for (k, co, ni, o16) in plan.calls[b]:
                        lo = k * BUCKET
                        hi = min(lo + BUCKET, ntot)
                        nc.gpsimd.dma_gather(
                            out_ap=G[:, co:co + ni // P, :],
                            in_ap=Mcur[lo:hi, :],
                            idxs_ap=idx_t[:, (o16 - o16b):(o16 - o16b) + ni // 16],
                            num_idxs=ni, num_idxs_reg=ni,
                            elem_size=nhid, queue_num=qrot % NQ,
                        )
                        qrot += 1

                    axps = psax.tile([P, nhid], FP32, tag="ax")
                    for j in range(ch):
                        segw = st.tile([P, P], BF16, tag="segw", bufs=4)
                        nc.vector.scalar_tensor_tensor(
                            out=segw[:], in0=iota_f[:],
                            scalar=ld_t[:, j:j + 1],
                            in1=w_t[:, j:j + 1].to_broadcast([P, P]),
                            op0=ALU.is_equal, op1=ALU.mult)
                        nc.tensor.matmul(axps[:], lhsT=segw[:], rhs=G[:, j, :],
                                         start=(j == 0), stop=(j == ch - 1))
                    ax = st.tile([P, nhid], FP32, tag="axs")
                    nc.scalar.activation(out=ax[:], in_=axps[:], func=AF.Copy)

                    # dense: accT += Wm^T @ AX^T
                    axt = st.tile([P, nh2, P], FP32, tag="axt")
                    for kc in range(nh2):
                        tps = ps.tile([P, P], FP32, tag="tr")
                        nc.tensor.transpose(out=tps[:],
                                            in_=ax[:, kc * P:(kc + 1) * P],
                                            identity=ident[:])
                        nc.scalar.activation(out=axt[:, kc, :], in_=tps[:],
                                             func=AF.Copy)
                    for m in range(nh2):
                        dps = ps.tile([P, P], FP32, tag="mm")
                        for kc in range(nh2):
                            nc.tensor.matmul(dps[:],
                                             lhsT=wm_sb[:, kc, m * P:(m + 1) * P],
                                             rhs=axt[:, kc, :],
                                             start=(kc == 0), stop=(kc == nh2 - 1))
                        nc.vector.tensor_add(out=accT[m][:, cols],
                                             in0=accT[m][:, cols], in1=dps[:])

                    if not last:
                        xc = st.tile([P, nhid], FP32, tag="xc")
                        nc.sync.dma_start(out=xc[:], in_=xprev[cols, :])
                        xn = st.tile([P, nhid], FP32, tag="xn")
                        nc.vector.tensor_sub(out=xn[:], in0=xc[:], in1=ax[:])
                        nc.vector.tensor_scalar_mul(out=xn[:], in0=xn[:],
                                                    scalar1=gm_sb[:, l:l + 1])
                        nc.scalar.dma_start(out=xnext[cols, :], in_=xn[:])
                        xnb = st.tile([P, nhid], BF16, tag="xnb")
                        nc.vector.tensor_copy(out=xnb[:], in_=xn[:])
                        nc.scalar.dma_start(out=agin[cols, :], in_=xnb[:])

                if not last:
                    nc.gpsimd.collective_compute(
                        "AllGather", ALU.bypass,
                        replica_groups=[list(range(CORES))],
                        ins=[agin[:]], outs=[Mnxt[:]],
                    )

            # ---------- classifier + log_softmax ----------
            for b in range(nb):
                cols = slice(b * P, (b + 1) * P)
                cps = ps.tile([ncls, P], FP32, tag="mm")
                for h in range(nh2):
                    nc.tensor.matmul(cps[:], lhsT=ws_sb[:, h, :],
                                     rhs=accT[h][:, cols],
                                     start=(h == 0), stop=(h == nh2 - 1))
                ot = st.tile([ncls, P], FP32, tag="ot")
                nc.vector.tensor_scalar(out=ot[:], in0=cps[:],
                                        scalar1=bs_sb[:], scalar2=None,
                                        op0=ALU.add)
                tps = ps.tile([P, ncls], FP32, tag="tr")
                nc.tensor.transpose(out=tps[:], in_=ot[:],
                                    identity=ident[:ncls, :ncls])
                lg = st.tile([P, ncls], FP32, tag="lg")
                nc.scalar.activation(out=lg[:], in_=tps[:], func=AF.Copy)
                nmx = st.tile([P, 1], FP32, tag="nmx")
                nc.vector.tensor_reduce(out=nmx[:], in_=lg[:],
                                        axis=mybir.AxisListType.X,
                                        op=ALU.max, negate=True)
                ex = st.tile([P, ncls], FP32, tag="ex")
                se = st.tile([P, 1], FP32, tag="se")
                nc.scalar.activation(out=ex[:], in_=lg[:], func=AF.Exp,
                                     bias=nmx[:], scale=1.0, accum_out=se[:])
                lz = st.tile([P, 1], FP32, tag="lz")
                nc.scalar.activation(out=lz[:], in_=se[:], func=AF.Ln)
                ob = st.tile([P, ncls], FP32, tag="ob")
                nc.vector.scalar_tensor_tensor(
                    out=ob[:], in0=lg[:], scalar=nmx[:],
                    in1=lz[:].to_broadcast([P, ncls]),
                    op0=ALU.add, op1=ALU.subtract)
                nc.sync.dma_start(out=out[cols, :], in_=ob[:])

    nc.compile()
    return nc


def run(plan: Plan, X, W_init, b_init, gammas, Ws_l, W_sort, b_sort,
        trace=False):
    nc = build_nc(plan)

    nl, nsh, nhid, ncls, nfeat = plan.nl, plan.nsh, plan.nhid, plan.ncls, plan.nfeat
    nh2 = nhid // P
    betas = TAU / np.arange(1, nl + 1, dtype=np.float64)
    eye = np.eye(nhid, dtype=np.float64)
    wm = np.stack([(1.0 - betas[i]) * eye + betas[i] * Ws_l[i].astype(np.float64)
                   for i in range(nl)]).astype(np.float32)
    bi2 = b_init.reshape(nh2, P).T.astype(np.float32).copy()      # [P, nh2]
    gm2 = np.tile(gammas.astype(np.float32)[None, :], (P, 1))     # [P, nl]
    bs2 = b_sort.astype(np.float32).reshape(ncls, 1)

    in_maps = []
    for c in range(CORES):
        xs = np.zeros((nfeat, nsh), np.float32)
        xs[:, :plan.nsh_raw] = X[c * plan.nsh_raw:(c + 1) * plan.nsh_raw].T
        in_maps.append({
            "xt": np.ascontiguousarray(xs),
            "wi": np.ascontiguousarray(W_init.astype(np.float32)),
            "bi": bi2, "wm": wm, "gm": gm2,
            "ws": np.ascontiguousarray(W_sort.astype(np.float32)),
            "bs": bs2,
            "eidx": np.ascontiguousarray(plan.eidx[c]),
            "eld": np.ascontiguousarray(plan.eld[c]),
            "ew": np.ascontiguousarray(plan.ew[c]),
        })

    res = bass_utils.run_bass_kernel_spmd(
        nc, in_maps, core_ids=list(range(CORES)),
        trace=trace, trace_cores=[0] if trace else None)

    outs = [res.results[c]["out"][:plan.nsh_raw] for c in range(CORES)]
    return np.concatenate(outs, axis=0), res


def kernel(X, edge_row, edge_col, edge_w, W_init, b_init, gammas, Ws,
           W_sort, b_sort):
    X = np.asarray(X)
    plan = Plan(100000, 512, 256, 64, 8,
                np.asarray(edge_row).astype(np.int64),
                np.asarray(edge_col).astype(np.int64),
                np.asarray(edge_w).astype(np.float32))
    out, _ = run(plan, X, np.asarray(W_init), np.asarray(b_init),
                 np.asarray(gammas), np.asarray(Ws), np.asarray(W_sort),
                 np.asarray(b_sort))
    return out.astype(np.float32)



# revision 4
# speedup vs baseline: 1.1802x; 1.1802x over previous
"""DRC-GCN (8-layer GNN message passing) on 8 trn2 NeuronCores — V2.

Strategy (node/data parallel, gather-based spmm):
  - Destination nodes sharded across 8 cores (12500 -> padded 12544 rows).
  - gamma factors are folded out of the recursion on the host:
      u_0 = H,  u_{l+1} = (I - A) u_l,   X_l^true = Gamma_{l-1} * u_l
    so the device only computes u_l and A·u_l.  The classifier is folded
    into every layer:  Z += (A u_l) @ P'_l  with  P'_l = Gamma_{l-1} *
    ((1-beta_l) I + beta_l W_l) @ W_sort  precomputed in float64 on host.
    Final phase is just log_softmax(Z + b_sort).
  - M_l = fp8(u_l) is AllGather'd per layer in 5 node-chunks (pipelined
    with compute); gathers for source-bucket p depend only on chunk-p's
    collective.  Gather rows are 256 B (fp8), one dma_gather call per
    (dest block, source bucket), capacities <= 1024 idxs.
  - spmm per dest block of 128: staircase of [128x128] selection matmuls
    (SegW bf16, one-hot * w) against gathered G (fp8) accumulating in PSUM.
    SegW is built 8 chunks per pair of DVE ops via broadcast APs.
  - Z kept as ZT [64, nsh] fp32 in SBUF; u kept as Xc [128, nb, 256] bf16
    in SBUF.
"""

import numpy as np
import ml_dtypes

import concourse.bass as bass
import concourse.mybir as mybir
import concourse.tile as tile
from concourse import bacc
from concourse import bass_utils

FP32 = mybir.dt.float32
BF16 = mybir.dt.bfloat16
FP8 = mybir.dt.float8e4
I16 = mybir.dt.int16
I32 = mybir.dt.int32
AF = mybir.ActivationFunctionType
ALU = mybir.AluOpType

P = 128
CORES = 8
TAU = 0.5
NCHUNK = 5          # comm chunks per layer == source buckets
NI_MAX = 1024       # dma_gather num_idxs hard limit on trn2
NQ = 4              # SWDGE queues
GB = 8              # SegW chunks built per DVE op pair
GBUFS = 4           # G tile double-buffer depth (blocks in flight)


def _ru(x, m):
    return (x + m - 1) // m * m


class Plan:
    """Static kernel structure + per-core data arrays."""

    def __init__(self, n_nodes, nfeat, nhid, ncls, nlayers,
                 edge_row, edge_col, edge_w):
        self.n = n_nodes
        self.nfeat = nfeat
        self.nhid = nhid
        self.ncls = ncls
        self.nl = nlayers
        assert n_nodes % CORES == 0
        self.nsh_raw = n_nodes // CORES
        self.nsh = _ru(self.nsh_raw, P)
        self.nb = self.nsh // P

        nch = min(NCHUNK, self.nb)
        self.nbuck = nch
        splits = np.array_split(np.arange(self.nb), nch)
        self.cb = np.array([len(s) for s in splits])            # blocks/chunk
        self.block_chunk = np.concatenate(
            [np.full(len(s), i) for i, s in enumerate(splits)])
        self.chunk_b0 = np.concatenate([[0], np.cumsum(self.cb)])  # block starts
        self.cs = self.cb * P                                   # rows/chunk/core
        self.rs = self.chunk_b0 * P                             # local row starts
        self.Bp = self.cs * CORES                               # bucket rows
        assert self.Bp.max() <= 32767

        # ---- edge mapping ----
        c_d = edge_row // self.nsh_raw
        r_d = edge_row - c_d * self.nsh_raw
        blk = r_d // P
        ld = r_d - blk * P
        c_s = edge_col // self.nsh_raw
        r_s = edge_col - c_s * self.nsh_raw
        blk_s = r_s // P
        p_s = self.block_chunk[blk_s]
        idx16 = c_s * self.cs[p_s] + (r_s - self.rs[p_s])

        counts = np.zeros((CORES, self.nb, nch), np.int64)
        np.add.at(counts, (c_d, blk, p_s), 1)
        cap = counts.max(axis=0)                                # [nb, nch]
        ni = np.where(cap > 0, _ru(np.maximum(cap, 1), 16), 0)  # call idxs
        assert ni.max() <= NI_MAX, f"segment cap {ni.max()} > {NI_MAX}"
        nchk = -(-ni // P)                                      # chunks/segment
        self.ch = nchk.sum(axis=1)                              # chunks/block
        self.chmax = int(self.ch.max())
        self.oc = np.concatenate([[0], np.cumsum(self.ch)]).astype(np.int64)
        self.totch = int(self.oc[-1])

        # call list per block: (bucket, chunk_off_in_block, ni, o16_off)
        self.calls = []
        o16 = 0
        for b in range(self.nb):
            cblk = []
            co = 0
            for k in range(nch):
                if ni[b, k] == 0:
                    continue
                cblk.append((k, co, int(ni[b, k]), o16))
                co += int(nchk[b, k])
                o16 += int(ni[b, k]) // 16
            self.calls.append(cblk)
        self.tot16 = o16

        # ---- per-core data arrays ----
        order = np.lexsort((idx16, p_s, blk, c_d))
        s_c, s_b, s_p = c_d[order], blk[order], p_s[order]
        s_i, s_ld, s_w = idx16[order], ld[order], edge_w[order]

        self.eidx = np.zeros((CORES, P, self.tot16), np.int16)
        self.eld = np.full((CORES, P, self.totch), -1.0, ml_dtypes.bfloat16)
        self.ew = np.zeros((CORES, P, self.totch), ml_dtypes.bfloat16)

        key = (s_c * self.nb + s_b) * nch + s_p
        starts = np.searchsorted(key, np.arange(CORES * self.nb * nch + 1))
        for c in range(CORES):
            for b in range(self.nb):
                for (k, co, nik, o16b) in self.calls[b]:
                    a0 = starts[(c * self.nb + b) * nch + k]
                    a1 = starts[(c * self.nb + b) * nch + k + 1]
                    nreal = a1 - a0
                    nsl = _ru(nik, P)
                    iv = np.zeros(nik, np.int16)
                    iv[:nreal] = s_i[a0:a1]
                    lv = np.full(nsl, -1.0, ml_dtypes.bfloat16)
                    lv[:nreal] = s_ld[a0:a1].astype(ml_dtypes.bfloat16)
                    wv = np.zeros(nsl, ml_dtypes.bfloat16)
                    wv[:nreal] = s_w[a0:a1].astype(ml_dtypes.bfloat16)
                    jj = np.arange(nsl)
                    ocol = self.oc[b] + co + jj // P
                    self.eld[c, jj % P, ocol] = lv
                    self.ew[c, jj % P, ocol] = wv
                    wr = iv.reshape(nik // 16, 16).T        # [16, nik/16]
                    self.eidx[c, :, o16b:o16b + nik // 16] = np.tile(wr, (8, 1))


def build_nc(plan: Plan):
    nl, nb, nsh = plan.nl, plan.nb, plan.nsh
    nhid, ncls, nfeat = plan.nhid, plan.ncls, plan.nfeat
    nh2 = nhid // P          # hidden partition-halves (2)
    nf2 = nfeat // P         # feat chunks (4)
    nch = plan.nbuck

    nc = bacc.Bacc("TRN2", target_bir_lowering=False, debug=False,
                   enable_asserts=True, num_devices=CORES, num_swdge_queues=NQ)

    xt = nc.dram_tensor("xt", [nfeat, nsh], BF16, kind="ExternalInput").ap()
    wi = nc.dram_tensor("wi", [nfeat, nhid], BF16, kind="ExternalInput").ap()
    bi = nc.dram_tensor("bi", [P, nh2], FP32, kind="ExternalInput").ap()
    ws = nc.dram_tensor("ws", [nhid, ncls], BF16, kind="ExternalInput").ap()
    bs = nc.dram_tensor("bs", [ncls, 1], FP32, kind="ExternalInput").ap()
    pl = nc.dram_tensor("pl", [nhid, nl, ncls], BF16, kind="ExternalInput").ap()
    eidx = nc.dram_tensor("eidx", [P, plan.tot16], I16, kind="ExternalInput").ap()
    eld = nc.dram_tensor("eld", [P, plan.totch], BF16, kind="ExternalInput").ap()
    ew = nc.dram_tensor("ew", [P, plan.totch], BF16, kind="ExternalInput").ap()
    out = nc.dram_tensor("out", [nsh, ncls], FP32, kind="ExternalOutput").ap()

    with tile.TileContext(nc) as tc:
        with tc.tile_pool(name="c1", bufs=1) as c1, \
             tc.tile_pool(name="stream", bufs=3) as st, \
             tc.tile_pool(name="meta", bufs=6) as mt, \
             tc.tile_pool(name="gpool", bufs=GBUFS) as gp, \
             tc.tile_pool(name="swpool", bufs=4) as swp, \
             tc.tile_pool(name="ps", bufs=2, space="PSUM") as ps, \
             tc.tile_pool(name="psax", bufs=2, space="PSUM") as psax, \
             tc.tile_pool(name="dram", bufs=1, space="DRAM") as dp:

            # ---------- constants ----------
            iota_i = c1.tile([P, P], I32)
            nc.gpsimd.iota(iota_i[:], pattern=[[1, P]], base=0,
                           channel_multiplier=0)
            iotap_i = c1.tile([P, 1], I32)
            nc.gpsimd.iota(iotap_i[:], pattern=[[1, 1]], base=0,
                           channel_multiplier=1)
            iota_f = c1.tile([P, P], FP32)
            nc.vector.tensor_copy(out=iota_f[:], in_=iota_i[:])
            iotap_f = c1.tile([P, 1], FP32)
            nc.vector.tensor_copy(out=iotap_f[:], in_=iotap_i[:])
            ident = c1.tile([P, P], FP32)
            nc.vector.tensor_tensor(out=ident[:], in0=iota_f[:],
                                    in1=iotap_f[:].to_broadcast([P, P]),
                                    op=ALU.is_equal)
            ident_bf = c1.tile([P, P], BF16)
            nc.vector.tensor_copy(out=ident_bf[:], in_=ident[:])
            # iota repeated GB times along a middle dim, bf16
            iota_rep = c1.tile([P, GB, P], BF16)
            for g in range(GB):
                nc.vector.tensor_copy(out=iota_rep[:, g, :], in_=iota_i[:])

            wi_sb = c1.tile([P, nf2, nhid], BF16)
            for kc in range(nf2):
                nc.sync.dma_start(out=wi_sb[:, kc, :],
                                  in_=wi[kc * P:(kc + 1) * P, :])
            bi_sb = c1.tile([P, nh2], FP32)
            nc.sync.dma_start(out=bi_sb[:], in_=bi[:])
            ws_sb = c1.tile([P, nh2, ncls], BF16)
            for h in range(nh2):
                nc.sync.dma_start(out=ws_sb[:, h, :],
                                  in_=ws[h * P:(h + 1) * P, :])
            bs_sb = c1.tile([ncls, 1], FP32)
            nc.sync.dma_start(out=bs_sb[:], in_=bs[:])
            pl_sb = c1.tile([P, nl * nh2, ncls], BF16)
            for l in range(nl):
                for kc in range(nh2):
                    nc.sync.dma_start(out=pl_sb[:, l * nh2 + kc, :],
                                      in_=pl[kc * P:(kc + 1) * P, l, :])

            # persistent state
            ZT = c1.tile([ncls, nsh], FP32, name="ZT")
            Xc = c1.tile([P, nb, nhid], BF16, name="Xc")

            # zero G buffers once (garbage slots must be finite: SegW row 0
            # times Inf/NaN would poison PSUM)
            for _ in range(GBUFS):
                g0 = gp.tile([P, plan.chmax, nhid], FP8, tag="g")
                nc.vector.memset(g0[:], 0.0)

            # DRAM intermediates
            Ms = [[dp.tile([int(plan.Bp[p]), nhid], FP8, addr_space="Shared",
                           name=f"M{l}_{p}") for p in range(nch)]
                  for l in range(nl)]
            ag = [[dp.tile([int(plan.cs[p]), nhid], FP8, name=f"ag{l}_{p}")
                   for p in range(nch)] for l in range(nl)]

            chunk_last = set((int(plan.chunk_b0[p + 1]) - 1, p)
                             for p in range(nch))

            def chunk_of(b):
                return int(plan.block_chunk[b])

            # ---------- init: u0 = H = X @ Wi + bi ; Z = H @ Ws + bs ----------
            for b in range(nb):
                cols = slice(b * P, (b + 1) * P)
                xt_sb = st.tile([P, nf2, P], BF16, tag="xt")
                for kc in range(nf2):
                    nc.sync.dma_start(out=xt_sb[:, kc, :],
                                      in_=xt[kc * P:(kc + 1) * P, cols])
                ht = st.tile([P, nh2, P], BF16, tag="ht")
                for h in range(nh2):
                    hps = ps.tile([P, P], FP32, tag="mm")
                    for kc in range(nf2):
                        nc.tensor.matmul(hps[:],
                                         lhsT=wi_sb[:, kc, h * P:(h + 1) * P],
                                         rhs=xt_sb[:, kc, :],
                                         start=(kc == 0), stop=(kc == nf2 - 1))
                    nc.scalar.activation(out=ht[:, h, :], in_=hps[:],
                                         func=AF.Identity,
                                         bias=bi_sb[:, h:h + 1])
                zps = ps.tile([ncls, P], FP32, tag="mm")
                for h in range(nh2):
                    nc.tensor.matmul(zps[:], lhsT=ws_sb[:, h, :],
                                     rhs=ht[:, h, :],
                                     start=(h == 0), stop=(h == nh2 - 1))
                nc.scalar.activation(out=ZT[:, cols], in_=zps[:],
                                     func=AF.Identity, bias=bs_sb[:])
                agst = st.tile([P, nhid], FP8, tag="agst")
                for h in range(nh2):
                    tps = ps.tile([P, P], BF16, tag="tr")
                    nc.tensor.transpose(out=tps[:], in_=ht[:, h, :],
                                        identity=ident_bf[:])
                    nc.scalar.activation(out=Xc[:, b, h * P:(h + 1) * P],
                                         in_=tps[:], func=AF.Copy)
                    nc.vector.tensor_copy(out=agst[:, h * P:(h + 1) * P],
                                          in_=tps[:])
                p = chunk_of(b)
                rows = slice((b - int(plan.chunk_b0[p])) * P,
                             (b - int(plan.chunk_b0[p])) * P + P)
                nc.scalar.dma_start(out=ag[0][p][rows, :], in_=agst[:])
                if (b, p) in chunk_last:
                    nc.gpsimd.collective_compute(
                        "AllGather", ALU.bypass,
                        replica_groups=[list(range(CORES))],
                        ins=[ag[0][p][:]], outs=[Ms[0][p][:]])

            # ---------- layers ----------
            qrot = 0
            for l in range(nl):
                last = (l == nl - 1)
                for b in range(nb):
                    cols = slice(b * P, (b + 1) * P)
                    ch = int(plan.ch[b])
                    oc = int(plan.oc[b])
                    calls = plan.calls[b]
                    o16b = calls[0][3] if calls else 0
                    n16 = sum(nik // 16 for (_, _, nik, _) in calls)

                    idx_t = mt.tile([P, n16], I16, tag="idx")
                    nc.sync.dma_start(out=idx_t[:],
                                      in_=eidx[:, o16b:o16b + n16])
                    ld_t = mt.tile([P, ch], BF16, tag="ld")
                    nc.sync.dma_start(out=ld_t[:], in_=eld[:, oc:oc + ch])
                    w_t = mt.tile([P, ch], BF16, tag="w")
                    nc.sync.dma_start(out=w_t[:], in_=ew[:, oc:oc + ch])

                    G = gp.tile([P, plan.chmax, nhid], FP8, tag="g")
                    for (k, co, nik, o16) in calls:
                        nc.gpsimd.dma_gather(
                            out_ap=G[:, co:co + _ru(nik, P) // P, :],
                            in_ap=Ms[l][k][:],
                            idxs_ap=idx_t[:, (o16 - o16b):(o16 - o16b) + nik // 16],
                            num_idxs=nik, num_idxs_reg=nik,
                            elem_size=nhid, queue_num=qrot % NQ)
                        qrot += 1

                    axps = psax.tile([P, nhid], FP32, tag="ax")
                    for g0 in range(0, ch, GB):
                        gn = min(GB, ch - g0)
                        oh = swp.tile([P, GB, P], BF16, tag="oh")
                        nc.vector.tensor_tensor(
                            out=oh[:, :gn, :], in0=iota_rep[:, :gn, :],
                            in1=ld_t[:, g0:g0 + gn].to_broadcast([P, gn, P]),
                            op=ALU.is_equal)
                        sw = swp.tile([P, GB, P], BF16, tag="sw")
                        nc.vector.tensor_tensor(
                            out=sw[:, :gn, :], in0=oh[:, :gn, :],
                            in1=w_t[:, g0:g0 + gn].to_broadcast([P, gn, P]),
                            op=ALU.mult)
                        for j in range(g0, g0 + gn):
                            nc.tensor.matmul(axps[:], lhsT=sw[:, j - g0, :],
                                             rhs=G[:, j, :],
                                             start=(j == 0), stop=(j == ch - 1))

                    ax = st.tile([P, nhid], BF16, tag="ax")
                    nc.scalar.activation(out=ax[:], in_=axps[:], func=AF.Copy)

                    # ZT[:, cols] += P'_l^T @ AX^T
                    axt = st.tile([P, nh2, P], BF16, tag="axt")
                    for kc in range(nh2):
                        tps = ps.tile([P, P], BF16, tag="tr")
                        nc.tensor.transpose(out=tps[:],
                                            in_=ax[:, kc * P:(kc + 1) * P],
                                            identity=ident_bf[:])
                        nc.scalar.activation(out=axt[:, kc, :], in_=tps[:],
                                             func=AF.Copy)
                    zps = ps.tile([ncls, P], FP32, tag="mm")
                    for kc in range(nh2):
                        nc.tensor.matmul(zps[:],
                                         lhsT=pl_sb[:, l * nh2 + kc, :],
                                         rhs=axt[:, kc, :],
                                         start=(kc == 0), stop=(kc == nh2 - 1))
                    nc.vector.tensor_add(out=ZT[:, cols], in0=ZT[:, cols],
                                         in1=zps[:])

                    if not last:
                        # u <- u - A u   (in place, bf16)
                        nc.vector.tensor_sub(out=Xc[:, b, :], in0=Xc[:, b, :],
                                             in1=ax[:])
                        agst = st.tile([P, nhid], FP8, tag="agst")
                        nc.scalar.activation(out=agst[:], in_=Xc[:, b, :],
                                             func=AF.Copy)
                        p = chunk_of(b)
                        rows = slice((b - int(plan.chunk_b0[p])) * P,
                                     (b - int(plan.chunk_b0[p])) * P + P)
                        nc.scalar.dma_start(out=ag[l + 1][p][rows, :],
                                            in_=agst[:])
                        if (b, p) in chunk_last:
                            nc.gpsimd.collective_compute(
                                "AllGather", ALU.bypass,
                                replica_groups=[list(range(CORES))],
                                ins=[ag[l + 1][p][:]], outs=[Ms[l + 1][p][:]])

            # ---------- log_softmax(ZT^T) ----------
            for b in range(nb):
                cols = slice(b * P, (b + 1) * P)
                tps = ps.tile([P, ncls], FP32, tag="mm")
                nc.tensor.transpose(out=tps[:], in_=ZT[:, cols],
                                    identity=ident[:ncls, :ncls])
                lg = st.tile([P, ncls], FP32, tag="lg")
                nc.scalar.activation(out=lg[:], in_=tps[:], func=AF.Copy)
                nmx = st.tile([P, 1], FP32, tag="nmx")
                nc.vector.tensor_reduce(out=nmx[:], in_=lg[:],
                                        axis=mybir.AxisListType.X,
                                        op=ALU.max, negate=True)
                ex = st.tile([P, ncls], FP32, tag="ex")
                se = st.tile([P, 1], FP32, tag="se")
                nc.scalar.activation(out=ex[:], in_=lg[:], func=AF.Exp,
                                     bias=nmx[:], scale=1.0, accum_out=se[:])
                lz = st.tile([P, 1], FP32, tag="lz")
                nc.scalar.activation(out=lz[:], in_=se[:], func=AF.Ln)
                ob = st.tile([P, ncls], FP32, tag="ob")
                nc.vector.scalar_tensor_tensor(
                    out=ob[:], in0=lg[:], scalar=nmx[:],
                    in1=lz[:].to_broadcast([P, ncls]),
                    op0=ALU.add, op1=ALU.subtract)
                nc.sync.dma_start(out=out[cols, :], in_=ob[:])

    nc.compile()
    return nc


def run(plan: Plan, X, W_init, b_init, gammas, Ws_l, W_sort, b_sort,
        trace=False):
    nc = build_nc(plan)

    nl, nsh, nhid, ncls, nfeat = plan.nl, plan.nsh, plan.nhid, plan.ncls, plan.nfeat
    nh2 = nhid // P
    betas = TAU / np.arange(1, nl + 1, dtype=np.float64)
    eye = np.eye(nhid, dtype=np.float64)
    gam = np.concatenate([[1.0], np.cumprod(gammas.astype(np.float64))])
    # P'_l = Gamma_{l-1} * ((1-b) I + b W_l) @ W_sort  -> [nhid, nl, ncls]
    plm = np.stack([
        gam[l] * (((1.0 - betas[l]) * eye + betas[l] * Ws_l[l].astype(np.float64))
                  @ W_sort.astype(np.float64))
        for l in range(nl)], axis=1).astype(ml_dtypes.bfloat16)
    bi2 = b_init.reshape(nh2, P).T.astype(np.float32).copy()      # [P, nh2]
    bs2 = b_sort.astype(np.float32).reshape(ncls, 1)

    in_maps = []
    for c in range(CORES):
        xs = np.zeros((nfeat, nsh), ml_dtypes.bfloat16)
        xs[:, :plan.nsh_raw] = X[c * plan.nsh_raw:(c + 1) * plan.nsh_raw].T
        in_maps.append({
            "xt": np.ascontiguousarray(xs),
            "wi": np.ascontiguousarray(W_init.astype(ml_dtypes.bfloat16)),
            "bi": bi2,
            "ws": np.ascontiguousarray(W_sort.astype(ml_dtypes.bfloat16)),
            "bs": bs2,
            "pl": np.ascontiguousarray(plm),
            "eidx": np.ascontiguousarray(plan.eidx[c]),
            "eld": np.ascontiguousarray(plan.eld[c]),
            "ew": np.ascontiguousarray(plan.ew[c]),
        })

    res = bass_utils.run_bass_kernel_spmd(
        nc, in_maps, core_ids=list(range(CORES)),
        trace=trace, trace_cores=[0] if trace else None)

    outs = [res.results[c]["out"][:plan.nsh_raw] for c in range(CORES)]
    return np.concatenate(outs, axis=0), res


def kernel(X, edge_row, edge_col, edge_w, W_init, b_init, gammas, Ws,
           W_sort, b_sort):
    X = np.asarray(X)
    plan = Plan(100000, 512, 256, 64, 8,
                np.asarray(edge_row).astype(np.int64),
                np.asarray(edge_col).astype(np.int64),
                np.asarray(edge_w).astype(np.float32))
    out, _ = run(plan, X, np.asarray(W_init), np.asarray(b_init),
                 np.asarray(gammas), np.asarray(Ws), np.asarray(W_sort),
                 np.asarray(b_sort))
    return out.astype(np.float32)
